# revision 2
# baseline (speedup 1.0000x reference)
"""Cross-attention Trainium2 kernel: build, host prep/gather, emulation.

Sharding: 8 cores = 4 batches x 2 head-halves. Core c=(b,j) computes
heads j*8..j*8+8 for batch b, producing a partial out.T [C, N]; host
sums the two partials per batch and adds bias.

All matmuls run in float32r (TF32-class, ~1.5e-4 rel err, full PE speed).
Contraction dims sit on SBUF partitions via host-side transposes:
  qT[o,n] = wqT.T @ xT ; kT[o,m] = wkT.T @ cT ; v[m,o] = cT.T @ wvT
  RoPE: pair-partner lives 16 partitions away inside each 32-partition
    quadrant (host permutes W columns accordingly) so one DVE
    stream_shuffle fetches it; q' = q*cos + shuf(q)*sin.
  S.T tile [m,n] = kT_h.T @ qT_h  (K=64)
  expS = ACT Exp(scale*S) PSUM->SBUF
  psO[65,n] += [v_h|1].T @ expS   (row 64 = softmax denominator)
  attnT *= 1/denom (partition-broadcast); outT[e,n] = wpT.T @ attnT
"""

import sys

sys.path.insert(0, "/opt/trn_rl_repo")

import numpy as np

import concourse.bass as bass
import concourse.tile as tile
from concourse import bacc, mybir
from concourse.bass_utils import run_bass_kernel_spmd

P = 128
SHUFFLE_MASK = [(i + 16) % 32 for i in range(32)]
F32 = mybir.dt.float32
F32R = mybir.dt.float32r


class CFG:
    def __init__(self, N=2048, M=2048, use_f32r=True):
        self.N, self.M = N, M
        self.C = 1024
        self.H = 16
        self.D = 64
        self.O = 512           # local head dim total (8 heads x 64)
        self.CC = self.C // P  # 8 c-chunks
        self.OC4 = self.O // P  # 4 o-chunks
        self.HPC = 8           # heads per core
        self.NPB = min(512, N)   # proj n-block
        self.NBL = min(1024, N)  # attention n-block
        self.use_f32r = use_f32r
        self.scale = self.D ** -0.5


def perm64():
    """Device partition row p (within a head's 64) -> original component."""
    out = []
    for p in range(64):
        q2, i = divmod(p, 32)
        pair = q2 * 16 + (i % 16)
        out.append(2 * pair + (0 if i < 16 else 1))
    return np.array(out)


def rope_tables(fc, L):
    """cos/sin tables [128, L] matching the permuted q/k layout."""
    cos = np.empty((P, L), np.float32)
    sin = np.empty((P, L), np.float32)
    for p in range(P):
        p64 = p % 64
        pair = (p64 // 32) * 16 + (p64 % 16)
        is_even = (p64 % 32) < 16
        cos[p] = fc[:L, pair, 0]
        sin[p] = fc[:L, pair, 1] * (-1.0 if is_even else 1.0)
    return cos, sin


def host_prep(x, context, freqs_cis, Wq, Wkv, Wproj, cfg):
    """Returns list of 8 in_maps."""
    N, M, C, O = cfg.N, cfg.M, cfg.C, cfg.O
    pr = perm64()
    cosq, sinq = rope_tables(freqs_cis, N)
    cosk, sink = rope_tables(freqs_cis, M)
    idx = np.concatenate([h * 64 + pr for h in range(cfg.HPC)])

    in_maps = []
    for core in range(8):
        b, j = divmod(core, 2)
        wq = Wq[j * O:(j + 1) * O, :][idx]
        wk = Wkv[j * O:(j + 1) * O, :][idx]
        wv = Wkv[C + j * O:C + (j + 1) * O, :]
        m = {
            "xT": np.ascontiguousarray(x[b].T),            # [C, N]
            "cT": np.ascontiguousarray(context[b].T),      # [C, M]
            "wqT": np.ascontiguousarray(wq.T),             # [C, O]
            "wkT": np.ascontiguousarray(wk.T),
            "wvT": np.ascontiguousarray(wv.T),
            "wpT": np.ascontiguousarray(Wproj[:, j * O:(j + 1) * O].T),  # [O, C]
            "cosq": cosq, "sinq": sinq,
        }
        if not (N == M):
            m["cosk"], m["sink"] = cosk, sink
        in_maps.append(m)
    return in_maps


def host_gather(results, bproj, cfg):
    outs = []
    for b in range(4):
        p0 = results[2 * b]["outT"]
        p1 = results[2 * b + 1]["outT"]
        outs.append((p0 + p1).T + bproj[None, :])
    return np.stack(outs).astype(np.float32)


def build_nc(cfg):
    N, M, C, O = cfg.N, cfg.M, cfg.C, cfg.O
    CC, OC4, HPC = cfg.CC, cfg.OC4, cfg.HPC
    NPB, NBL = cfg.NPB, cfg.NBL
    n_pb, m_pb = N // NPB, M // NPB
    MC = M // P
    NH = N // NBL
    RDT = F32R if cfg.use_f32r else F32

    nc = bacc.Bacc("TRN2", target_bir_lowering=False, debug=False)
    xT = nc.dram_tensor("xT", [C, N], RDT, kind="ExternalInput").ap()
    cT = nc.dram_tensor("cT", [C, M], RDT, kind="ExternalInput").ap()
    wqT = nc.dram_tensor("wqT", [C, O], RDT, kind="ExternalInput").ap()
    wkT = nc.dram_tensor("wkT", [C, O], RDT, kind="ExternalInput").ap()
    wvT = nc.dram_tensor("wvT", [C, O], RDT, kind="ExternalInput").ap()
    wpT = nc.dram_tensor("wpT", [O, C], RDT, kind="ExternalInput").ap()
    cosq = nc.dram_tensor("cosq", [P, N], F32, kind="ExternalInput").ap()
    sinq = nc.dram_tensor("sinq", [P, N], F32, kind="ExternalInput").ap()
    if N == M:
        cosk, sink = cosq, sinq
    else:
        cosk = nc.dram_tensor("cosk", [P, M], F32, kind="ExternalInput").ap()
        sink = nc.dram_tensor("sink", [P, M], F32, kind="ExternalInput").ap()
    outT = nc.dram_tensor("outT", [C, N], F32, kind="ExternalOutput").ap()

    Exp = mybir.ActivationFunctionType.Exp
    dma = nc.sync.dma_start

    with tile.TileContext(nc) as tc:
        with tc.tile_pool(name="persist", bufs=1) as pp:
            # ---- persistent tiles (128.5 KB/partition)
            qT = pp.tile([P, OC4, N], RDT, tag="qT")
            kT = pp.tile([P, OC4, M], RDT, tag="kT")
            vAll = pp.tile([P, MC, HPC * 65], RDT, tag="vAll")
            attnT = pp.tile([P, OC4, N], RDT, tag="attnT")
            nc.vector.memset(vAll[:, :, :].bitcast(F32), 1.0)

            # ================= phase 1: Q/K/V projections =================
            with (
                tc.tile_pool(name="wqkv", bufs=1) as wqkv_pool,
                tc.tile_pool(name="xc", bufs=4) as xc_pool,
                tc.tile_pool(name="psQ", bufs=5, space="PSUM") as psQ,
            ):
                wq_sb = wqkv_pool.tile([P, CC, O], RDT, tag="wq_sb")
                wk_sb = wqkv_pool.tile([P, CC, O], RDT, tag="wk_sb")
                wv_sb = wqkv_pool.tile([P, CC, O], RDT, tag="wv_sb")
                for w_sb, w_dram in ((wq_sb, wqT), (wk_sb, wkT), (wv_sb, wvT)):
                    dma(w_sb[:, :, :], w_dram.rearrange("(cc p) o -> p cc o", p=P))

                # Q projection: qT[o, n]
                for nb in range(n_pb):
                    ns = bass.ts(nb, NPB)
                    pss = [psQ.tile([P, NPB], F32, tag="ps_proj", name=f"psq{nb}_{i}")
                           for i in range(OC4)]
                    for cc in range(CC):
                        x_sb = xc_pool.tile([P, NPB], RDT, tag="x_sb")
                        dma(x_sb[:, :], xT[bass.ts(cc, P), ns])
                        for c4 in range(OC4):
                            nc.tensor.matmul(
                                pss[c4][:, :],
                                wq_sb[:, cc, bass.ts(c4, P)],
                                x_sb[:, :],
                                start=(cc == 0), stop=(cc == CC - 1),
                            )
                    for c4 in range(OC4):
                        nc.vector.tensor_copy(qT[:, c4, ns], pss[c4][:, :])

                # K projection
                for mb in range(m_pb):
                    ms = bass.ts(mb, NPB)
                    pss = [psQ.tile([P, NPB], F32, tag="ps_proj", name=f"psk{mb}_{i}")
                           for i in range(OC4)]
                    for cc in range(CC):
                        c_sb = xc_pool.tile([P, NPB], RDT, tag="x_sb")
                        dma(c_sb[:, :], cT[bass.ts(cc, P), ms])
                        for c4 in range(OC4):
                            nc.tensor.matmul(
                                pss[c4][:, :],
                                wk_sb[:, cc, bass.ts(c4, P)],
                                c_sb[:, :],
                                start=(cc == 0), stop=(cc == CC - 1),
                            )
                    for c4 in range(OC4):
                        nc.vector.tensor_copy(kT[:, c4, ms], pss[c4][:, :])

                # V projection: v[m, o] with per-head ones column
                for mb in range(m_pb):
                    ms = bass.ts(mb, NPB)
                    n_mc2 = NPB // P
                    pss = [psQ.tile([P, O], F32, tag="ps_proj", name=f"psv{mb}_{i}")
                           for i in range(n_mc2)]
                    for cc in range(CC):
                        c_sb = xc_pool.tile([P, NPB], RDT, tag="x_sb")
                        dma(c_sb[:, :], cT[bass.ts(cc, P), ms])
                        for mc2 in range(n_mc2):
                            nc.tensor.matmul(
                                pss[mc2][:, :],
                                c_sb[:, bass.ts(mc2, P)],
                                wv_sb[:, cc, :],
                                start=(cc == 0), stop=(cc == CC - 1),
                            )
                    for mc2 in range(n_mc2):
                        mc = mb * n_mc2 + mc2
                        nc.vector.tensor_copy(
                            vAll[:, mc, :].rearrange("p (h e) -> p h e", e=65)[:, :, 0:64],
                            pss[mc2][:, :].rearrange("p (h d) -> p h d", d=64),
                        )

            # ===== phases 2+3: RoPE interleaved with attention ===========
            with (
                tc.tile_pool(name="ctab", bufs=1) as ctab_pool,
                tc.tile_pool(name="rope", bufs=2) as rope_pool,
                tc.tile_pool(name="exps", bufs=3) as exp_pool,
                tc.tile_pool(name="recipp", bufs=2) as recip_pool,
                tc.tile_pool(name="psS", bufs=2, space="PSUM") as psS_pool,
                tc.tile_pool(name="psO", bufs=2, space="PSUM") as psO_pool,
            ):
                cos_q = ctab_pool.tile([P, N], F32, tag="cos_q")
                sin_q = ctab_pool.tile([P, N], F32, tag="sin_q")
                if N == M:
                    cos_k, sin_k = cos_q, sin_q
                else:
                    cos_k = ctab_pool.tile([P, M], F32, tag="cos_k")
                    sin_k = ctab_pool.tile([P, M], F32, tag="sin_k")
                dma(cos_q[:, :], cosq)
                dma(sin_q[:, :], sinq)
                if N != M:
                    dma(cos_k[:, :], cosk)
                    dma(sin_k[:, :], sink)

                RB = min(1024, N)

                def rope_chunk(t, cos_t, sin_t, L, c4):
                    for rb in range(L // RB):
                        rs = bass.ts(rb, RB)
                        sw = rope_pool.tile([P, RB], F32, tag="rope_sw", name="sw")
                        nc.vector.stream_shuffle(
                            sw[:, :], t[:, c4, rs].bitcast(F32), SHUFFLE_MASK)
                        t1 = rope_pool.tile([P, RB], F32, tag="rope_t1", name="t1")
                        nc.vector.tensor_mul(t1[:, :], t[:, c4, rs], cos_t[:, rs])
                        nc.vector.tensor_mul(sw[:, :], sw[:, :], sin_t[:, rs])
                        nc.vector.tensor_add(t[:, c4, rs], t1[:, :], sw[:, :])

                def emit_qk(rows, c4, nh, mc):
                    psS = psS_pool.tile([P, NBL], F32, tag="psS", name="psS")
                    for q in range(NBL // 512):
                        # stop=False: skip the per-matmul accumulation-group
                        # flush; exp reads the region after completion anyway
                        nc.tensor.matmul(
                            psS[:, bass.ts(q, 512)],
                            kT[rows, c4, bass.ts(mc, P)],
                            qT[rows, c4, bass.ds(nh * NBL + q * 512, 512)],
                            start=True, stop=False, skip_group_check=True,
                        )
                    return psS

                for c4 in range(OC4):
                    rope_chunk(qT, cos_q, sin_q, N, c4)
                    rope_chunk(kT, cos_k, sin_k, M, c4)
                    for hh in range(2):
                        h = 2 * c4 + hh
                        rows = slice(hh * 64, hh * 64 + 64)
                        for nh in range(NH):
                            nsl = bass.ts(nh, NBL)
                            psO = psO_pool.tile([65, NBL], F32, tag="psO", name="psO")
                            # software pipeline: QK one mc ahead of exp/PV so
                            # the in-order PE queue never blocks on ACT
                            psS_cur = emit_qk(rows, c4, nh, 0)
                            for mc in range(MC):
                                psS_nxt = (emit_qk(rows, c4, nh, mc + 1)
                                           if mc + 1 < MC else None)
                                eS = exp_pool.tile([P, NBL], RDT, tag="eS", name="eS")
                                nc.scalar.activation(eS[:, :], psS_cur[:, :], Exp,
                                                     scale=cfg.scale)
                                for q in range(NBL // 512):
                                    nc.tensor.matmul(
                                        psO[:, bass.ts(q, 512)],
                                        vAll[:, mc, bass.ds(h * 65, 65)],
                                        eS[:, bass.ts(q, 512)],
                                        start=(mc == 0), stop=(mc == MC - 1),
                                    )
                                psS_cur = psS_nxt
                            # fused normalize + evict
                            rc = recip_pool.tile([1, NBL], F32, tag="rc", name="rc")
                            nc.vector.reciprocal(rc[:, :], psO[64:65, :])
                            rb = recip_pool.tile([64, NBL], F32, tag="rb", name="rb")
                            nc.gpsimd.partition_broadcast(rb[:, :], rc[:, :])
                            nc.vector.tensor_mul(attnT[rows, c4, nsl], psO[0:64, :],
                                                 rb[:, :])

            # ================= phase 4: output projection =================
            with (
                tc.tile_pool(name="wpp", bufs=1) as wpp_pool,
                tc.tile_pool(name="oev", bufs=3) as oev_pool,
                tc.tile_pool(name="psP", bufs=3, space="PSUM") as psP_pool,
            ):
                wp_sb = wpp_pool.tile([P, OC4, C], RDT, tag="wp_sb")
                dma(wp_sb[:, :, :], wpT.rearrange("(oc p) e -> p oc e", p=P))
                for nb in range(n_pb):
                    ns = bass.ts(nb, NPB)
                    for ec in range(C // P):
                        ps = psP_pool.tile([P, NPB], F32, tag="ps_out")
                        for oc in range(OC4):
                            nc.tensor.matmul(
                                ps[:, :],
                                wp_sb[:, oc, bass.ts(ec, P)],
                                attnT[:, oc, ns],
                                start=(oc == 0), stop=(oc == OC4 - 1),
                            )
                        ot = oev_pool.tile([P, NPB], F32, tag="ot")
                        nc.vector.tensor_copy(ot[:, :], ps[:, :])
                        dma(outT[bass.ts(ec, P), ns], ot[:, :])

    nc.compile()
    return nc


# ---------------------------------------------------------------- emulation
def emulate_core(m, cfg):
    """Numpy replica of the device program (layout validation)."""
    N, M, C, O = cfg.N, cfg.M, cfg.C, cfg.O
    xT, cT = m["xT"], m["cT"]
    qT = (m["wqT"].T @ xT)
    kT = (m["wkT"].T @ cT)
    v = (cT.T @ m["wvT"])
    cosk = m.get("cosk", m["cosq"])
    sink = m.get("sink", m["sinq"])

    def rope(tT, cos, sin, L):
        t = tT.reshape(cfg.OC4, P, L)
        out = np.empty_like(t)
        for c4 in range(cfg.OC4):
            blk = t[c4]
            sw = np.empty_like(blk)
            for s in range(4):
                for i in range(32):
                    sw[s * 32 + i] = blk[s * 32 + SHUFFLE_MASK[i]]
            out[c4] = blk * cos + sw * sin
        return out.reshape(O, L)

    qT = rope(qT, m["cosq"], m["sinq"], N)
    kT = rope(kT, cosk, sink, M)

    attnT = np.empty((O, N), np.float32)
    for h in range(cfg.HPC):
        qh = qT[h * 64:(h + 1) * 64, :]
        kh = kT[h * 64:(h + 1) * 64, :]
        S = kh.T @ qh
        E = np.exp(cfg.scale * S)
        vh = v[:, h * 64:(h + 1) * 64]
        num = vh.T @ E
        den = E.sum(axis=0)
        attnT[h * 64:(h + 1) * 64] = num / den[None, :]
    return m["wpT"].T @ attnT


# ---------------------------------------------------------------- driver
_NC_CACHE = {}


def _get_nc(cfg):
    key = (cfg.N, cfg.M, cfg.use_f32r)
    if key not in _NC_CACHE:
        _NC_CACHE[key] = build_nc(cfg)
    return _NC_CACHE[key]


def _run(inputs, trace=False):
    cfg = CFG()
    nc = _get_nc(cfg)
    in_maps = host_prep(
        np.asarray(inputs["x"], np.float32),
        np.asarray(inputs["context"], np.float32),
        np.asarray(inputs["freqs_cis"], np.float32),
        np.asarray(inputs["Wq"], np.float32),
        np.asarray(inputs["Wkv"], np.float32),
        np.asarray(inputs["Wproj"], np.float32),
        cfg,
    )
    res = run_bass_kernel_spmd(nc, in_maps, list(range(8)), trace=trace)
    out = host_gather(res.results, np.asarray(inputs["bproj"], np.float32), cfg)
    return out, res


def kernel(**inputs):
    out, _ = _run(inputs, trace=False)
    return out


def timed_run(inputs):
    _, res = _run(inputs, trace=True)
    return res.exec_time_ns, res



# revision 21
# speedup vs baseline: 1.5901x; 1.5901x over previous
"""Cross-attention Trainium2 kernel: build, host prep/gather, emulation.

Sharding: 8 cores = 4 batches x 2 head-halves. Core c=(b,j) computes
heads j*8..j*8+8 for batch b, producing a partial out.T [C, N]; host
sums the two partials per batch and adds bias.

All matmuls run in fp16 (PSUM accumulation f32). Contraction dims sit
on SBUF partitions via host-side transposes:
  qT[o,n] = wqT.T @ xT ; kT[o,m] = wkT.T @ cT ; v[m,o] = cT.T @ wvT
  RoPE: pair-partner lives 16 partitions away inside each 32-partition
    quadrant (host permutes W columns accordingly) so one DVE
    stream_shuffle (on a uint32 view) fetches it; q' = q*cos + shuf(q)*sin.
  Attention runs per head PAIR (heads 2c4, 2c4+1 live on partitions
    0-63 / 64-127 of the c4 block): the two K=64 S matmuls occupy
    disjoint PE row-groups (tile_position auto (0,0)/(64,0)) and run
    CONCURRENTLY in the array.
  S.T tile [m,n] = kT_h.T @ qT_h  (K=64, NBL=512 wide)
  expS: head1 chunk on ACT (Exp activation), head2 chunk on DVE via the
    Schraudolph bit-trick writing bf16 bits through an int16 cast:
    bf16_bits(e^x) ~= int16(184.665*x + 16250.75). Both engines run in
    parallel with PE; a few head1 chunks are stolen to DVE to balance.
  eS/vAll are bf16 (exp spans e^-13..e^13 -- fp16 would overflow);
    the q/k/S path and attnT/wp are fp16 for mantissa precision.
  psO[65,n] += [v_h|1].T @ expS   (row 64 = softmax denominator)
  attnT = psO * recip(den) (reciprocal_approx_fast + gpsimd broadcast)
  outT[e,n] = wpT.T @ attnT
"""

import sys

sys.path.insert(0, "/opt/trn_rl_repo")

import numpy as np
import ml_dtypes

import concourse.bass as bass
import concourse.tile as tile
from concourse import bacc, mybir
from concourse.bass_utils import run_bass_kernel_spmd

P = 128
SHUFFLE_MASK = [(i + 16) % 32 for i in range(32)]
F32 = mybir.dt.float32
F16 = mybir.dt.float16
BF16 = mybir.dt.bfloat16
I16 = mybir.dt.int16
U32 = mybir.dt.uint32
F16NP = np.float16
BFNP = ml_dtypes.bfloat16

# bf16 Schraudolph: bf16_bits(e^x) ~= int16(EXP_A*x + EXP_B)
# (bf16 target: exponent covers |x| up to ~30 -- scores reach |x|~13)
EXP_A = 184.6649652337873
EXP_B = 16250.75
# mc chunks whose head2 exp runs on ACT instead of DVE (load balance:
# DVE also carries rope/normalize, so ACT takes 20 of 32 chunks)
ACT_H2_MCS = frozenset({3, 7, 11, 15})


class CFG:
    def __init__(self, N=2048, M=2048):
        self.N, self.M = N, M
        self.C = 1024
        self.H = 16
        self.D = 64
        self.O = 512           # local head dim total (8 heads x 64)
        self.CC = self.C // P  # 8 c-chunks
        self.OC4 = self.O // P  # 4 o-chunks
        self.HPC = 8           # heads per core
        self.NPB = min(512, N)   # proj n-block
        self.NBL = min(512, N)   # attention n-block
        self.scale = self.D ** -0.5


def perm64():
    """Device partition row p (within a head's 64) -> original component."""
    out = []
    for p in range(64):
        q2, i = divmod(p, 32)
        pair = q2 * 16 + (i % 16)
        out.append(2 * pair + (0 if i < 16 else 1))
    return np.array(out)


def rope_tables(fc, L):
    """cos/sin tables [128, L] matching the permuted q/k layout."""
    cos = np.empty((P, L), np.float32)
    sin = np.empty((P, L), np.float32)
    for p in range(P):
        p64 = p % 64
        pair = (p64 // 32) * 16 + (p64 % 16)
        is_even = (p64 % 32) < 16
        cos[p] = fc[:L, pair, 0]
        sin[p] = fc[:L, pair, 1] * (-1.0 if is_even else 1.0)
    return cos, sin


def host_prep(x, context, freqs_cis, Wq, Wkv, Wproj, cfg):
    """Returns list of 8 in_maps (bf16 device layouts)."""
    N, M, C, O = cfg.N, cfg.M, cfg.C, cfg.O
    pr = perm64()
    cosq, sinq = rope_tables(freqs_cis, N)
    cosk, sink = rope_tables(freqs_cis, M)
    idx = np.concatenate([h * 64 + pr for h in range(cfg.HPC)])

    def b16(a):
        return np.ascontiguousarray(a).astype(F16NP)

    in_maps = []
    for core in range(8):
        b, j = divmod(core, 2)
        wq = Wq[j * O:(j + 1) * O, :][idx]
        wk = Wkv[j * O:(j + 1) * O, :][idx]
        wv = Wkv[C + j * O:C + (j + 1) * O, :]
        m = {
            "xT": b16(x[b].T),            # [C, N]
            "cT": b16(context[b].T),      # [C, M]
            "wqT": b16(wq.T),             # [C, O]
            "wkT": b16(wk.T),
            "wvT": b16(wv.T),
            "wpT": b16(Wproj[:, j * O:(j + 1) * O].T),  # [O, C]
            "cosq": b16(cosq), "sinq": b16(sinq),
        }
        if not (N == M):
            m["cosk"], m["sink"] = b16(cosk), b16(sink)
        in_maps.append(m)
    return in_maps


def host_gather(results, bproj, cfg):
    outs = []
    for b in range(4):
        p0 = results[2 * b]["outT"]
        p1 = results[2 * b + 1]["outT"]
        outs.append((np.asarray(p0) + np.asarray(p1)).T + bproj[None, :])
    return np.stack(outs).astype(np.float32)


def build_nc(cfg):
    N, M, C, O = cfg.N, cfg.M, cfg.C, cfg.O
    CC, OC4, HPC = cfg.CC, cfg.OC4, cfg.HPC
    NPB, NBL = cfg.NPB, cfg.NBL
    n_pb, m_pb = N // NPB, M // NPB
    MC = M // P
    NH = N // NBL

    nc = bacc.Bacc("TRN2", target_bir_lowering=False, debug=False)
    xT = nc.dram_tensor("xT", [C, N], F16, kind="ExternalInput").ap()
    cT = nc.dram_tensor("cT", [C, M], F16, kind="ExternalInput").ap()
    wqT = nc.dram_tensor("wqT", [C, O], F16, kind="ExternalInput").ap()
    wkT = nc.dram_tensor("wkT", [C, O], F16, kind="ExternalInput").ap()
    wvT = nc.dram_tensor("wvT", [C, O], F16, kind="ExternalInput").ap()
    wpT = nc.dram_tensor("wpT", [O, C], F16, kind="ExternalInput").ap()
    cosq = nc.dram_tensor("cosq", [P, N], F16, kind="ExternalInput").ap()
    sinq = nc.dram_tensor("sinq", [P, N], F16, kind="ExternalInput").ap()
    if N == M:
        cosk, sink = cosq, sinq
    else:
        cosk = nc.dram_tensor("cosk", [P, M], F16, kind="ExternalInput").ap()
        sink = nc.dram_tensor("sink", [P, M], F16, kind="ExternalInput").ap()
    outT = nc.dram_tensor("outT", [C, N], F32, kind="ExternalOutput").ap()

    Exp = mybir.ActivationFunctionType.Exp
    Mult = mybir.AluOpType.mult
    Add = mybir.AluOpType.add
    dma = nc.sync.dma_start
    A_dve = EXP_A * cfg.scale

    with tile.TileContext(nc) as tc:
        with tc.tile_pool(name="persist", bufs=1) as pp:
            # ---- persistent tiles (~64.3 KB/partition)
            qT = pp.tile([P, OC4, N], F16, tag="qT")
            kT = pp.tile([P, OC4, M], F16, tag="kT")
            vAll = pp.tile([P, MC, HPC * 65], BF16, tag="vAll")
            attnT = pp.tile([P, OC4, N], F16, tag="attnT")
            nc.vector.memset(vAll[:, :, :], 1.0)

            # ================= phase 1: Q/K/V projections =================
            with (
                tc.tile_pool(name="wqkv", bufs=1) as wqkv_pool,
                tc.tile_pool(name="xc", bufs=4) as xc_pool,
                tc.tile_pool(name="psQ", bufs=1, space="PSUM") as psQ,
            ):
                wq_sb = wqkv_pool.tile([P, CC, O], F16, tag="wq_sb")
                wk_sb = wqkv_pool.tile([P, CC, O], F16, tag="wk_sb")
                wv_sb = wqkv_pool.tile([P, CC, O], F16, tag="wv_sb")
                for w_sb, w_dram in ((wq_sb, wqT), (wk_sb, wkT), (wv_sb, wvT)):
                    dma(w_sb[:, :, :], w_dram.rearrange("(cc p) o -> p cc o", p=P))

                # Q projection: qT[o, n]
                for nb in range(n_pb):
                    ns = bass.ts(nb, NPB)
                    pss = [psQ.tile([P, NPB], F32, tag=f"psq{i}",
                                    name=f"psq{nb}_{i}")
                           for i in range(OC4)]
                    for cc in range(CC):
                        x_sb = xc_pool.tile([P, NPB], F16, tag="x_sb")
                        dma(x_sb[:, :], xT[bass.ts(cc, P), ns])
                        for c4 in range(OC4):
                            nc.tensor.matmul(
                                pss[c4][:, :],
                                wq_sb[:, cc, bass.ts(c4, P)],
                                x_sb[:, :],
                                start=(cc == 0), stop=(cc == CC - 1),
                            )
                    for c4 in range(OC4):
                        nc.scalar.copy(qT[:, c4, ns], pss[c4][:, :])

                # K+V projections fused (share context loads)
                n_mc2 = NPB // P
                for mb in range(m_pb):
                    ms = bass.ts(mb, NPB)
                    psk = [psQ.tile([P, NPB], F32, tag=f"psq{i}",
                                    name=f"psk{mb}_{i}")
                           for i in range(OC4)]
                    psv = [psQ.tile([P, O], F32, tag=f"psv{i}",
                                    name=f"psv{mb}_{i}")
                           for i in range(n_mc2)]
                    for cc in range(CC):
                        c_sb = xc_pool.tile([P, NPB], F16, tag="x_sb")
                        dma(c_sb[:, :], cT[bass.ts(cc, P), ms])
                        for c4 in range(OC4):
                            nc.tensor.matmul(
                                psk[c4][:, :],
                                wk_sb[:, cc, bass.ts(c4, P)],
                                c_sb[:, :],
                                start=(cc == 0), stop=(cc == CC - 1),
                            )
                        for mc2 in range(n_mc2):
                            nc.tensor.matmul(
                                psv[mc2][:, :],
                                c_sb[:, bass.ts(mc2, P)],
                                wv_sb[:, cc, :],
                                start=(cc == 0), stop=(cc == CC - 1),
                            )
                    for c4 in range(OC4):
                        nc.scalar.copy(kT[:, c4, ms], psk[c4][:, :])
                    for mc2 in range(n_mc2):
                        mc = mb * n_mc2 + mc2
                        nc.vector.tensor_copy(
                            vAll[:, mc, :].rearrange("p (h e) -> p h e", e=65)[:, :, 0:64],
                            psv[mc2][:, :].rearrange("p (h d) -> p h d", d=64),
                        )

            # ===== phases 2+3: RoPE interleaved with attention ===========
            with (
                tc.tile_pool(name="ctab", bufs=1) as ctab_pool,
                tc.tile_pool(name="rope", bufs=2) as rope_pool,
                tc.tile_pool(name="exps", bufs=4) as exp_pool,
                tc.tile_pool(name="recipp", bufs=2) as recip_pool,
                tc.tile_pool(name="psS", bufs=2, space="PSUM") as psS_pool,
                tc.tile_pool(name="psO", bufs=2, space="PSUM") as psO_pool,
            ):
                cos_q = ctab_pool.tile([P, N], F16, tag="cos_q")
                sin_q = ctab_pool.tile([P, N], F16, tag="sin_q")
                if N == M:
                    cos_k, sin_k = cos_q, sin_q
                else:
                    cos_k = ctab_pool.tile([P, M], F16, tag="cos_k")
                    sin_k = ctab_pool.tile([P, M], F16, tag="sin_k")
                dma(cos_q[:, :], cosq)
                dma(sin_q[:, :], sinq)
                if N != M:
                    dma(cos_k[:, :], cosk)
                    dma(sin_k[:, :], sink)

                def rope_chunk(t, cos_t, sin_t, L, c4):
                    sw = rope_pool.tile([P, L], F16, tag="rope_sw", name="sw")
                    nc.vector.stream_shuffle(
                        sw[:, :].bitcast(U32), t[:, c4, :].bitcast(U32),
                        SHUFFLE_MASK)
                    t1 = rope_pool.tile([P, L], F16, tag="rope_t1", name="t1")
                    nc.vector.tensor_mul(t1[:, :], t[:, c4, :], cos_t[:, :])
                    nc.vector.tensor_mul(sw[:, :], sw[:, :], sin_t[:, :])
                    nc.vector.tensor_add(t[:, c4, :], t1[:, :], sw[:, :])

                def emit_S(c4, nh, mc):
                    """Two K=64 S matmuls on disjoint PE row-groups."""
                    nsl = bass.ds(nh * NBL, NBL)
                    msl = bass.ts(mc, P)
                    psS1 = psS_pool.tile([P, NBL], F32, tag="psS1", name="psS1")
                    psS2 = psS_pool.tile([P, NBL], F32, tag="psS2", name="psS2")
                    nc.tensor.matmul(psS1[:, :], kT[0:64, c4, msl],
                                     qT[0:64, c4, nsl], start=True, stop=True)
                    nc.tensor.matmul(psS2[:, :], kT[64:128, c4, msl],
                                     qT[64:128, c4, nsl], start=True, stop=True)
                    return psS1, psS2

                # rope everything up front (DVE overlaps V-proj tail; kills
                # the QKV->attention bubble and c4-transition bubbles)
                for c4 in range(OC4):
                    rope_chunk(qT, cos_q, sin_q, N, c4)
                    rope_chunk(kT, cos_k, sin_k, M, c4)

                for c4 in range(OC4):
                    h1, h2 = 2 * c4, 2 * c4 + 1
                    for nh in range(NH):
                        nsl = bass.ts(nh, NBL)
                        psO1 = psO_pool.tile([65, NBL], F32, tag="psO1",
                                             name="psO1")
                        psO2 = psO_pool.tile([65, NBL], F32, tag="psO2",
                                             name="psO2")
                        cur = emit_S(c4, nh, 0)
                        for mc in range(MC):
                            nxt = emit_S(c4, nh, mc + 1) if mc + 1 < MC else None
                            eS1 = exp_pool.tile([P, NBL], BF16, tag="eS1",
                                                name="eS1")
                            eS2 = exp_pool.tile([P, NBL], BF16, tag="eS2",
                                                name="eS2")
                            nc.scalar.activation(eS1[:, :], cur[0][:, :],
                                                 Exp, scale=cfg.scale)
                            if mc in ACT_H2_MCS:
                                nc.scalar.activation(eS2[:, :], cur[1][:, :],
                                                     Exp, scale=cfg.scale)
                            else:
                                nc.vector.tensor_scalar(
                                    eS2[:, :].bitcast(I16), cur[1][:, :],
                                    A_dve, EXP_B, Mult, Add)
                            nc.tensor.matmul(
                                psO1[:, :], vAll[:, mc, bass.ds(h1 * 65, 65)],
                                eS1[:, :], start=(mc == 0), stop=(mc == MC - 1),
                            )
                            nc.tensor.matmul(
                                psO2[:, :], vAll[:, mc, bass.ds(h2 * 65, 65)],
                                eS2[:, :], start=(mc == 0), stop=(mc == MC - 1),
                            )
                            cur = nxt
                        # fused normalize + evict (per head). den row sits on
                        # psO partition 64; reciprocal_approx_fast misreads
                        # nonzero base partitions, so stage den to partition 0
                        # via a scalar-engine copy first.
                        for hh, psO in ((0, psO1), (1, psO2)):
                            rows = slice(hh * 64, hh * 64 + 64)
                            dn = recip_pool.tile([1, NBL], F32, tag=f"dn{hh}",
                                                 name="dn")
                            nc.vector.tensor_copy(dn[:, :], psO[64:65, :])
                            rc = recip_pool.tile([1, NBL], F32, tag=f"rc{hh}",
                                                 name="rc")
                            nc.vector.reciprocal_approx_fast(rc[:, :],
                                                             dn[:, :])
                            rb = recip_pool.tile([64, NBL], F32, tag=f"rb{hh}",
                                                 name="rb")
                            nc.gpsimd.partition_broadcast(rb[:, :], rc[:, :])
                            nc.vector.tensor_mul(attnT[rows, c4, nsl],
                                                 psO[0:64, :], rb[:, :])

            # ================= phase 4: output projection =================
            with (
                tc.tile_pool(name="wpp", bufs=1) as wpp_pool,
                tc.tile_pool(name="oev", bufs=3) as oev_pool,
                tc.tile_pool(name="psP", bufs=3, space="PSUM") as psP_pool,
            ):
                wp_sb = wpp_pool.tile([P, OC4, C], F16, tag="wp_sb")
                dma(wp_sb[:, :, :], wpT.rearrange("(oc p) e -> p oc e", p=P))
                for nb in range(n_pb):
                    ns = bass.ts(nb, NPB)
                    for ec in range(C // P):
                        ps = psP_pool.tile([P, NPB], F32, tag="ps_out")
                        for oc in range(OC4):
                            nc.tensor.matmul(
                                ps[:, :],
                                wp_sb[:, oc, bass.ts(ec, P)],
                                attnT[:, oc, ns],
                                start=(oc == 0), stop=(oc == OC4 - 1),
                            )
                        ot = oev_pool.tile([P, NPB], F32, tag="ot")
                        nc.scalar.copy(ot[:, :], ps[:, :])
                        dma(outT[bass.ts(ec, P), ns], ot[:, :])

    nc.compile()
    return nc


# ---------------------------------------------------------------- emulation
def _bf(a):
    return np.asarray(a).astype(BFNP).astype(np.float32)


def _f16(a):
    return np.asarray(a).astype(np.float16).astype(np.float32)


def _schraudolph(x):
    """Emulate the DVE bf16 exp trick (truncating f32->i16 cast)."""
    i = np.floor(EXP_A * x + EXP_B).astype(np.int16)
    return i.view(BFNP).astype(np.float32)


def emulate_core(m, cfg):
    """Numpy replica of the device program (layout + numerics validation)."""
    N, M, C, O = cfg.N, cfg.M, cfg.C, cfg.O
    xT = _f16(m["xT"])
    cT = _f16(m["cT"])
    qT = _f16(_f16(m["wqT"]).T @ xT)
    kT = _f16(_f16(m["wkT"]).T @ cT)
    v = _bf(cT.T @ _f16(m["wvT"]))
    cosk = _f16(m.get("cosk", m["cosq"]))
    sink = _f16(m.get("sink", m["sinq"]))
    cosq_t, sinq_t = _f16(m["cosq"]), _f16(m["sinq"])

    def rope(tT, cos, sin, L):
        t = tT.reshape(cfg.OC4, P, L)
        out = np.empty_like(t)
        for c4 in range(cfg.OC4):
            blk = t[c4]
            sw = np.empty_like(blk)
            for s in range(4):
                for i in range(32):
                    sw[s * 32 + i] = blk[s * 32 + SHUFFLE_MASK[i]]
            out[c4] = _f16(_f16(blk * cos) + _f16(sw * sin))
        return out.reshape(O, L)

    qT = rope(qT, cosq_t, sinq_t, N)
    kT = rope(kT, cosk, sink, M)

    attnT = np.empty((O, N), np.float32)
    for h in range(cfg.HPC):
        qh = qT[h * 64:(h + 1) * 64, :]
        kh = kT[h * 64:(h + 1) * 64, :]
        S = kh.T @ qh
        use_dve = h % 2 == 1
        E = np.empty((M, N), np.float32)
        for mc in range(M // P):
            sl = slice(mc * P, (mc + 1) * P)
            if use_dve and mc not in ACT_H2_MCS:
                E[sl] = _schraudolph(cfg.scale * S[sl])
            else:
                E[sl] = _bf(np.exp(cfg.scale * S[sl]))
        vh = v[:, h * 64:(h + 1) * 64]
        num = vh.T @ E
        den = E.sum(axis=0)
        attnT[h * 64:(h + 1) * 64] = _f16(num / den[None, :])
    return _f16(m["wpT"]).T.astype(np.float32) @ attnT


# ---------------------------------------------------------------- driver
_NC_CACHE = {}


def _get_nc(cfg):
    key = (cfg.N, cfg.M)
    if key not in _NC_CACHE:
        _NC_CACHE[key] = build_nc(cfg)
    return _NC_CACHE[key]


def _run(inputs, trace=False):
    cfg = CFG()
    nc = _get_nc(cfg)
    in_maps = host_prep(
        np.asarray(inputs["x"], np.float32),
        np.asarray(inputs["context"], np.float32),
        np.asarray(inputs["freqs_cis"], np.float32),
        np.asarray(inputs["Wq"], np.float32),
        np.asarray(inputs["Wkv"], np.float32),
        np.asarray(inputs["Wproj"], np.float32),
        cfg,
    )
    res = run_bass_kernel_spmd(nc, in_maps, list(range(8)), trace=trace)
    out = host_gather(res.results, np.asarray(inputs["bproj"], np.float32), cfg)
    return out, res


def kernel(**inputs):
    out, _ = _run(inputs, trace=False)
    return out


def timed_run(inputs):
    _, res = _run(inputs, trace=True)
    return res.exec_time_ns, res


# revision 23
# speedup vs baseline: 1.6851x; 1.0597x over previous
"""Cross-attention Trainium2 kernel: build, host prep/gather, emulation.

Sharding: 8 cores = 4 batches x 2 head-halves. Core c=(b,j) computes
heads j*8..j*8+8 for batch b, producing a partial out.T [C, N]; host
sums the two partials per batch and adds bias.

All matmuls run in fp16 (PSUM accumulation f32). Contraction dims sit
on SBUF partitions via host-side transposes:
  qT[o,n] = wqT.T @ xT ; kT[o,m] = wkT.T @ cT ; v[m,o] = cT.T @ wvT
  RoPE: pair-partner lives 16 partitions away inside each 32-partition
    quadrant (host permutes W columns accordingly) so one DVE
    stream_shuffle (on a uint32 view) fetches it; q' = q*cos + shuf(q)*sin.
  Attention runs per head PAIR (heads 2c4, 2c4+1 live on partitions
    0-63 / 64-127 of the c4 block): the two K=64 S matmuls occupy
    disjoint PE row-groups (tile_position auto (0,0)/(64,0)) and run
    CONCURRENTLY in the array.
  S.T tile [m,n] = kT_h.T @ qT_h  (K=64, NBL=512 wide)
  expS: head1 chunk on ACT (Exp activation), head2 chunk on DVE via the
    Schraudolph bit-trick writing bf16 bits through an int16 cast:
    bf16_bits(e^x) ~= int16(184.665*x + 16250.75). Both engines run in
    parallel with PE; a few head1 chunks are stolen to DVE to balance.
  eS/vAll are bf16 (exp spans e^-13..e^13 -- fp16 would overflow);
    the q/k/S path and attnT/wp are fp16 for mantissa precision.
  psO[65,n] += [v_h|1].T @ expS   (row 64 = softmax denominator)
  attnT = psO * recip(den) (reciprocal_approx_fast + gpsimd broadcast)
  outT[e,n] = wpT.T @ attnT
"""

import sys

sys.path.insert(0, "/opt/trn_rl_repo")

import numpy as np
import ml_dtypes

import concourse.bass as bass
import concourse.tile as tile
from concourse import bacc, mybir
from concourse.bass_utils import run_bass_kernel_spmd

P = 128
SHUFFLE_MASK = [(i + 16) % 32 for i in range(32)]
F32 = mybir.dt.float32
F16 = mybir.dt.float16
BF16 = mybir.dt.bfloat16
I16 = mybir.dt.int16
U32 = mybir.dt.uint32
F16NP = np.float16
BFNP = ml_dtypes.bfloat16

# bf16 Schraudolph: bf16_bits(e^x) ~= int16(EXP_A*x + EXP_B)
# (bf16 target: exponent covers |x| up to ~30 -- scores reach |x|~13)
EXP_A = 184.6649652337873
EXP_B = 16250.75
# mc chunks whose head2 exp runs on ACT instead of DVE (load balance:
# DVE also carries rope/normalize, so ACT takes 20 of 32 chunks)
ACT_H2_MCS = frozenset({3, 7, 11, 15})


class CFG:
    def __init__(self, N=2048, M=2048):
        self.N, self.M = N, M
        self.C = 1024
        self.H = 16
        self.D = 64
        self.O = 512           # local head dim total (8 heads x 64)
        self.CC = self.C // P  # 8 c-chunks
        self.OC4 = self.O // P  # 4 o-chunks
        self.HPC = 8           # heads per core
        self.NPB = min(512, N)   # proj n-block
        self.NBL = min(512, N)   # attention n-block
        self.scale = self.D ** -0.5


def perm64():
    """Device partition row p (within a head's 64) -> original component."""
    out = []
    for p in range(64):
        q2, i = divmod(p, 32)
        pair = q2 * 16 + (i % 16)
        out.append(2 * pair + (0 if i < 16 else 1))
    return np.array(out)


def rope_tables(fc, L):
    """cos/sin tables [128, L] matching the permuted q/k layout."""
    cos = np.empty((P, L), np.float32)
    sin = np.empty((P, L), np.float32)
    for p in range(P):
        p64 = p % 64
        pair = (p64 // 32) * 16 + (p64 % 16)
        is_even = (p64 % 32) < 16
        cos[p] = fc[:L, pair, 0]
        sin[p] = fc[:L, pair, 1] * (-1.0 if is_even else 1.0)
    return cos, sin


def host_prep(x, context, freqs_cis, Wq, Wkv, Wproj, cfg):
    """Returns list of 8 in_maps (bf16 device layouts)."""
    N, M, C, O = cfg.N, cfg.M, cfg.C, cfg.O
    pr = perm64()
    cosq, sinq = rope_tables(freqs_cis, N)
    cosk, sink = rope_tables(freqs_cis, M)
    idx = np.concatenate([h * 64 + pr for h in range(cfg.HPC)])

    def b16(a):
        return np.ascontiguousarray(a).astype(F16NP)

    in_maps = []
    for core in range(8):
        b, j = divmod(core, 2)
        wq = Wq[j * O:(j + 1) * O, :][idx]
        wk = Wkv[j * O:(j + 1) * O, :][idx]
        wv = Wkv[C + j * O:C + (j + 1) * O, :]
        m = {
            "xT": b16(x[b].T),            # [C, N]
            "cT": b16(context[b].T),      # [C, M]
            "wqT": b16(wq.T),             # [C, O]
            "wkT": b16(wk.T),
            "wvT": b16(wv.T),
            "wpT": b16(Wproj[:, j * O:(j + 1) * O].T),  # [O, C]
            "cosq": b16(cosq), "sinq": b16(sinq),
        }
        if not (N == M):
            m["cosk"], m["sink"] = b16(cosk), b16(sink)
        in_maps.append(m)
    return in_maps


def host_gather(results, bproj, cfg):
    outs = []
    for b in range(4):
        p0 = results[2 * b]["outT"]
        p1 = results[2 * b + 1]["outT"]
        outs.append((np.asarray(p0) + np.asarray(p1)).T + bproj[None, :])
    return np.stack(outs).astype(np.float32)


def build_nc(cfg):
    N, M, C, O = cfg.N, cfg.M, cfg.C, cfg.O
    CC, OC4, HPC = cfg.CC, cfg.OC4, cfg.HPC
    NPB, NBL = cfg.NPB, cfg.NBL
    n_pb, m_pb = N // NPB, M // NPB
    MC = M // P
    NH = N // NBL

    nc = bacc.Bacc("TRN2", target_bir_lowering=False, debug=False)
    xT = nc.dram_tensor("xT", [C, N], F16, kind="ExternalInput").ap()
    cT = nc.dram_tensor("cT", [C, M], F16, kind="ExternalInput").ap()
    wqT = nc.dram_tensor("wqT", [C, O], F16, kind="ExternalInput").ap()
    wkT = nc.dram_tensor("wkT", [C, O], F16, kind="ExternalInput").ap()
    wvT = nc.dram_tensor("wvT", [C, O], F16, kind="ExternalInput").ap()
    wpT = nc.dram_tensor("wpT", [O, C], F16, kind="ExternalInput").ap()
    cosq = nc.dram_tensor("cosq", [P, N], F16, kind="ExternalInput").ap()
    sinq = nc.dram_tensor("sinq", [P, N], F16, kind="ExternalInput").ap()
    if N == M:
        cosk, sink = cosq, sinq
    else:
        cosk = nc.dram_tensor("cosk", [P, M], F16, kind="ExternalInput").ap()
        sink = nc.dram_tensor("sink", [P, M], F16, kind="ExternalInput").ap()
    outT = nc.dram_tensor("outT", [C, N], F32, kind="ExternalOutput").ap()

    Exp = mybir.ActivationFunctionType.Exp
    Mult = mybir.AluOpType.mult
    Add = mybir.AluOpType.add
    dma = nc.sync.dma_start
    A_dve = EXP_A * cfg.scale

    with tile.TileContext(nc) as tc:
        with tc.tile_pool(name="persist", bufs=1) as pp:
            # ---- persistent tiles (~64.3 KB/partition)
            qT = pp.tile([P, OC4, N], F16, tag="qT")
            kT = pp.tile([P, OC4, M], F16, tag="kT")
            vAll = pp.tile([P, MC, HPC * 65], BF16, tag="vAll")
            attnT = pp.tile([P, OC4, N], F16, tag="attnT")
            nc.vector.memset(vAll[:, :, :], 1.0)

            # ================= phase 1: Q/K/V projections =================
            with (
                tc.tile_pool(name="wqkv", bufs=1) as wqkv_pool,
                tc.tile_pool(name="xc", bufs=4) as xc_pool,
                tc.tile_pool(name="psQ", bufs=1, space="PSUM") as psQ,
            ):
                wq_sb = wqkv_pool.tile([P, CC, O], F16, tag="wq_sb")
                wk_sb = wqkv_pool.tile([P, CC, O], F16, tag="wk_sb")
                wv_sb = wqkv_pool.tile([P, CC, O], F16, tag="wv_sb")
                for w_sb, w_dram in ((wq_sb, wqT), (wk_sb, wkT), (wv_sb, wvT)):
                    dma(w_sb[:, :, :], w_dram.rearrange("(cc p) o -> p cc o", p=P))

                # Q projection: qT[o, n]
                for nb in range(n_pb):
                    ns = bass.ts(nb, NPB)
                    pss = [psQ.tile([P, NPB], F32, tag=f"psq{i}",
                                    name=f"psq{nb}_{i}")
                           for i in range(OC4)]
                    for cc in range(CC):
                        x_sb = xc_pool.tile([P, NPB], F16, tag="x_sb")
                        dma(x_sb[:, :], xT[bass.ts(cc, P), ns])
                        for c4 in range(OC4):
                            nc.tensor.matmul(
                                pss[c4][:, :],
                                wq_sb[:, cc, bass.ts(c4, P)],
                                x_sb[:, :],
                                start=(cc == 0), stop=(cc == CC - 1),
                            )
                    for c4 in range(OC4):
                        nc.scalar.copy(qT[:, c4, ns], pss[c4][:, :])

                # K+V projections fused (share context loads)
                n_mc2 = NPB // P
                for mb in range(m_pb):
                    ms = bass.ts(mb, NPB)
                    psk = [psQ.tile([P, NPB], F32, tag=f"psq{i}",
                                    name=f"psk{mb}_{i}")
                           for i in range(OC4)]
                    psv = [psQ.tile([P, O], F32, tag=f"psv{i}",
                                    name=f"psv{mb}_{i}")
                           for i in range(n_mc2)]
                    for cc in range(CC):
                        c_sb = xc_pool.tile([P, NPB], F16, tag="x_sb")
                        dma(c_sb[:, :], cT[bass.ts(cc, P), ms])
                        for c4 in range(OC4):
                            nc.tensor.matmul(
                                psk[c4][:, :],
                                wk_sb[:, cc, bass.ts(c4, P)],
                                c_sb[:, :],
                                start=(cc == 0), stop=(cc == CC - 1),
                            )
                        for mc2 in range(n_mc2):
                            nc.tensor.matmul(
                                psv[mc2][:, :],
                                c_sb[:, bass.ts(mc2, P)],
                                wv_sb[:, cc, :],
                                start=(cc == 0), stop=(cc == CC - 1),
                            )
                    for c4 in range(OC4):
                        nc.scalar.copy(kT[:, c4, ms], psk[c4][:, :])
                    for mc2 in range(n_mc2):
                        mc = mb * n_mc2 + mc2
                        nc.vector.tensor_copy(
                            vAll[:, mc, :].rearrange("p (h e) -> p h e", e=65)[:, :, 0:64],
                            psv[mc2][:, :].rearrange("p (h d) -> p h d", d=64),
                        )

            # ===== phases 2+3: RoPE interleaved with attention ===========
            with (
                tc.tile_pool(name="ctab", bufs=1) as ctab_pool,
                tc.tile_pool(name="rope", bufs=2) as rope_pool,
                tc.tile_pool(name="exps", bufs=4) as exp_pool,
                tc.tile_pool(name="recipp", bufs=2) as recip_pool,
                tc.tile_pool(name="psS", bufs=3, space="PSUM") as psS_pool,
                tc.tile_pool(name="psO", bufs=1, space="PSUM") as psO_pool,
            ):
                cos_q = ctab_pool.tile([P, N], F16, tag="cos_q")
                sin_q = ctab_pool.tile([P, N], F16, tag="sin_q")
                if N == M:
                    cos_k, sin_k = cos_q, sin_q
                else:
                    cos_k = ctab_pool.tile([P, M], F16, tag="cos_k")
                    sin_k = ctab_pool.tile([P, M], F16, tag="sin_k")
                dma(cos_q[:, :], cosq)
                dma(sin_q[:, :], sinq)
                if N != M:
                    dma(cos_k[:, :], cosk)
                    dma(sin_k[:, :], sink)

                def rope_chunk(t, cos_t, sin_t, L, c4):
                    sw = rope_pool.tile([P, L], F16, tag="rope_sw", name="sw")
                    nc.vector.stream_shuffle(
                        sw[:, :].bitcast(U32), t[:, c4, :].bitcast(U32),
                        SHUFFLE_MASK)
                    t1 = rope_pool.tile([P, L], F16, tag="rope_t1", name="t1")
                    nc.vector.tensor_mul(t1[:, :], t[:, c4, :], cos_t[:, :])
                    nc.vector.tensor_mul(sw[:, :], sw[:, :], sin_t[:, :])
                    nc.vector.tensor_add(t[:, c4, :], t1[:, :], sw[:, :])

                def emit_S(c4, nh, mc):
                    """Two K=64 S matmuls on disjoint PE row-groups."""
                    nsl = bass.ds(nh * NBL, NBL)
                    msl = bass.ts(mc, P)
                    psS1 = psS_pool.tile([P, NBL], F32, tag="psS1", name="psS1")
                    psS2 = psS_pool.tile([P, NBL], F32, tag="psS2", name="psS2")
                    nc.tensor.matmul(psS1[:, :], kT[0:64, c4, msl],
                                     qT[0:64, c4, nsl], start=True, stop=True)
                    nc.tensor.matmul(psS2[:, :], kT[64:128, c4, msl],
                                     qT[64:128, c4, nsl], start=True, stop=True)
                    return psS1, psS2

                # rope everything up front (DVE overlaps V-proj tail; kills
                # the QKV->attention bubble and c4-transition bubbles)
                for c4 in range(OC4):
                    rope_chunk(qT, cos_q, sin_q, N, c4)
                    rope_chunk(kT, cos_k, sin_k, M, c4)

                for c4 in range(OC4):
                    h1, h2 = 2 * c4, 2 * c4 + 1
                    for nh in range(NH):
                        nsl = bass.ts(nh, NBL)
                        psO1 = psO_pool.tile([65, NBL], F32, tag="psO1",
                                             name="psO1")
                        psO2 = psO_pool.tile([65, NBL], F32, tag="psO2",
                                             name="psO2")
                        # depth-2 software pipeline: S runs two mc ahead and
                        # exp(mc) is issued right after S(mc), so eS(mc) is
                        # ready long before PV(mc) -- the PE never stalls and
                        # stays in its ramped clock state.
                        def emit_exp(pair, mc):
                            eS1 = exp_pool.tile([P, NBL], BF16, tag="eS1",
                                                name="eS1")
                            eS2 = exp_pool.tile([P, NBL], BF16, tag="eS2",
                                                name="eS2")
                            nc.scalar.activation(eS1[:, :], pair[0][:, :],
                                                 Exp, scale=cfg.scale)
                            if mc in ACT_H2_MCS:
                                nc.scalar.activation(eS2[:, :], pair[1][:, :],
                                                     Exp, scale=cfg.scale)
                            else:
                                nc.vector.tensor_scalar(
                                    eS2[:, :].bitcast(I16), pair[1][:, :],
                                    A_dve, EXP_B, Mult, Add)
                            return eS1, eS2

                        es_q = [emit_exp(emit_S(c4, nh, 0), 0)]
                        if MC > 1:
                            es_q.append(emit_exp(emit_S(c4, nh, 1), 1))
                        for mc in range(MC):
                            if mc + 2 < MC:
                                es_q.append(
                                    emit_exp(emit_S(c4, nh, mc + 2), mc + 2))
                            eS1, eS2 = es_q.pop(0)
                            nc.tensor.matmul(
                                psO1[:, :], vAll[:, mc, bass.ds(h1 * 65, 65)],
                                eS1[:, :], start=(mc == 0), stop=(mc == MC - 1),
                            )
                            nc.tensor.matmul(
                                psO2[:, :], vAll[:, mc, bass.ds(h2 * 65, 65)],
                                eS2[:, :], start=(mc == 0), stop=(mc == MC - 1),
                            )
                        # fused normalize + evict (per head). den row sits on
                        # psO partition 64; reciprocal_approx_fast misreads
                        # nonzero base partitions, so stage den to partition 0
                        # via a scalar-engine copy first.
                        for hh, psO in ((0, psO1), (1, psO2)):
                            rows = slice(hh * 64, hh * 64 + 64)
                            dn = recip_pool.tile([1, NBL], F32, tag=f"dn{hh}",
                                                 name="dn")
                            nc.vector.tensor_copy(dn[:, :], psO[64:65, :])
                            rc = recip_pool.tile([1, NBL], F32, tag=f"rc{hh}",
                                                 name="rc")
                            nc.vector.reciprocal_approx_fast(rc[:, :],
                                                             dn[:, :])
                            rb = recip_pool.tile([64, NBL], F32, tag=f"rb{hh}",
                                                 name="rb")
                            nc.gpsimd.partition_broadcast(rb[:, :], rc[:, :])
                            nc.vector.tensor_mul(attnT[rows, c4, nsl],
                                                 psO[0:64, :], rb[:, :])

            # ================= phase 4: output projection =================
            with (
                tc.tile_pool(name="wpp", bufs=1) as wpp_pool,
                tc.tile_pool(name="oev", bufs=3) as oev_pool,
                tc.tile_pool(name="psP", bufs=3, space="PSUM") as psP_pool,
            ):
                wp_sb = wpp_pool.tile([P, OC4, C], F16, tag="wp_sb")
                dma(wp_sb[:, :, :], wpT.rearrange("(oc p) e -> p oc e", p=P))
                for nb in range(n_pb):
                    ns = bass.ts(nb, NPB)
                    for ec in range(C // P):
                        ps = psP_pool.tile([P, NPB], F32, tag="ps_out")
                        for oc in range(OC4):
                            nc.tensor.matmul(
                                ps[:, :],
                                wp_sb[:, oc, bass.ts(ec, P)],
                                attnT[:, oc, ns],
                                start=(oc == 0), stop=(oc == OC4 - 1),
                            )
                        ot = oev_pool.tile([P, NPB], F32, tag="ot")
                        nc.scalar.copy(ot[:, :], ps[:, :])
                        dma(outT[bass.ts(ec, P), ns], ot[:, :])

    nc.compile()
    return nc


# ---------------------------------------------------------------- emulation
def _bf(a):
    return np.asarray(a).astype(BFNP).astype(np.float32)


def _f16(a):
    return np.asarray(a).astype(np.float16).astype(np.float32)


def _schraudolph(x):
    """Emulate the DVE bf16 exp trick (truncating f32->i16 cast)."""
    i = np.floor(EXP_A * x + EXP_B).astype(np.int16)
    return i.view(BFNP).astype(np.float32)


def emulate_core(m, cfg):
    """Numpy replica of the device program (layout + numerics validation)."""
    N, M, C, O = cfg.N, cfg.M, cfg.C, cfg.O
    xT = _f16(m["xT"])
    cT = _f16(m["cT"])
    qT = _f16(_f16(m["wqT"]).T @ xT)
    kT = _f16(_f16(m["wkT"]).T @ cT)
    v = _bf(cT.T @ _f16(m["wvT"]))
    cosk = _f16(m.get("cosk", m["cosq"]))
    sink = _f16(m.get("sink", m["sinq"]))
    cosq_t, sinq_t = _f16(m["cosq"]), _f16(m["sinq"])

    def rope(tT, cos, sin, L):
        t = tT.reshape(cfg.OC4, P, L)
        out = np.empty_like(t)
        for c4 in range(cfg.OC4):
            blk = t[c4]
            sw = np.empty_like(blk)
            for s in range(4):
                for i in range(32):
                    sw[s * 32 + i] = blk[s * 32 + SHUFFLE_MASK[i]]
            out[c4] = _f16(_f16(blk * cos) + _f16(sw * sin))
        return out.reshape(O, L)

    qT = rope(qT, cosq_t, sinq_t, N)
    kT = rope(kT, cosk, sink, M)

    attnT = np.empty((O, N), np.float32)
    for h in range(cfg.HPC):
        qh = qT[h * 64:(h + 1) * 64, :]
        kh = kT[h * 64:(h + 1) * 64, :]
        S = kh.T @ qh
        use_dve = h % 2 == 1
        E = np.empty((M, N), np.float32)
        for mc in range(M // P):
            sl = slice(mc * P, (mc + 1) * P)
            if use_dve and mc not in ACT_H2_MCS:
                E[sl] = _schraudolph(cfg.scale * S[sl])
            else:
                E[sl] = _bf(np.exp(cfg.scale * S[sl]))
        vh = v[:, h * 64:(h + 1) * 64]
        num = vh.T @ E
        den = E.sum(axis=0)
        attnT[h * 64:(h + 1) * 64] = _f16(num / den[None, :])
    return _f16(m["wpT"]).T.astype(np.float32) @ attnT


# ---------------------------------------------------------------- driver
_NC_CACHE = {}


def _get_nc(cfg):
    key = (cfg.N, cfg.M)
    if key not in _NC_CACHE:
        _NC_CACHE[key] = build_nc(cfg)
    return _NC_CACHE[key]


def _run(inputs, trace=False):
    cfg = CFG()
    nc = _get_nc(cfg)
    in_maps = host_prep(
        np.asarray(inputs["x"], np.float32),
        np.asarray(inputs["context"], np.float32),
        np.asarray(inputs["freqs_cis"], np.float32),
        np.asarray(inputs["Wq"], np.float32),
        np.asarray(inputs["Wkv"], np.float32),
        np.asarray(inputs["Wproj"], np.float32),
        cfg,
    )
    res = run_bass_kernel_spmd(nc, in_maps, list(range(8)), trace=trace)
    out = host_gather(res.results, np.asarray(inputs["bproj"], np.float32), cfg)
    return out, res


def kernel(**inputs):
    out, _ = _run(inputs, trace=False)
    return out


def timed_run(inputs):
    _, res = _run(inputs, trace=True)
    return res.exec_time_ns, res
